# revision 2
# baseline (speedup 1.0000x reference)
"""Trainium2 Bass kernel for nn_MultiHeadCoAttention.

B=32, LT=512, LI=576, D=768, H=8, K=512. Batch-parallel over 8 cores
(4 batches per core, no collectives). All matmuls in float32r.

v2 changes vs baseline:
- DRAM tensors declared float32r; DMA lands directly in matmul-ready
  tiles (no stage buffer, no gpsimd rounding pass).
- Tiled loads batched into single 3D-AP DMA descriptors.
- image/text row layouts loaded once per batch, reused by the G_v
  passes and the context matmuls.
- PSUM->SBUF copies run on gpsimd (Pool) instead of DVE.
"""
import sys
sys.path.insert(0, '/opt/trn_rl_repo')
import numpy as np
import concourse.bacc as bacc
import concourse.tile as tile
from concourse import bass, mybir
from concourse.bass_utils import run_bass_kernel_spmd

F32 = mybir.dt.float32
F32R = mybir.dt.float32r
BF16 = mybir.dt.bfloat16
AF = mybir.ActivationFunctionType
OP = mybir.AluOpType

B, LT, LI, D, H, K = 32, 512, 576, 768, 8, 512
NB = 4           # batches per core
N_CORES = 8
ET = D // 128    # 6 e-tiles
XT = LT // 128   # 4 x-tiles
YT = 5           # y-tiles (4 full + 1 of 64)
LA = D + 4       # aug row width (768 + [1, 0, 0, 0])


def ycols(j):
    return 128 if j < 4 else 64


def build_nc(repeat=1, opts=None):
    o = dict(tail_sbuf=True, mult_pool=True, add_pool=False,
             red_pool=False, copy_split=True,
             hch_bufs=2, prod_bufs=2, psC_bufs=2, wbt_bufs=2,
             wbt_half=True, inp2=False, wqq2=False, wob=2,
             dma_split=True, wot_bufs=5, wo_bf16=True, wo_pool_dma=True)
    o.update(opts or {})
    nc = bacc.Bacc(None, target_bir_lowering=False)

    # ---- DRAM I/O (per core) ----
    textT = nc.dram_tensor("textT", [NB, D, LT], F32R, kind="ExternalInput")
    text_aug = nc.dram_tensor("text_aug", [NB, LT, LA], F32R, kind="ExternalInput")
    imageT = nc.dram_tensor("imageT", [NB, D, LI], F32R, kind="ExternalInput")
    image_aug = nc.dram_tensor("image_aug", [NB, LI, LA], F32R, kind="ExternalInput")
    WqT_d = nc.dram_tensor("WqT", [D, K], F32R, kind="ExternalInput")
    WvT_d = nc.dram_tensor("WvT", [D, K], F32R, kind="ExternalInput")
    WbT_d = nc.dram_tensor("WbT", [H, D, D], F32R, kind="ExternalInput")
    WhvB_d = nc.dram_tensor("WhvB", [128, K], F32, kind="ExternalInput")
    WhqB_d = nc.dram_tensor("WhqB", [128, K], F32, kind="ExternalInput")
    wo_dt = BF16 if o["wo_bf16"] else F32R
    WoT_d = nc.dram_tensor("WoT", [H * D, D], wo_dt, kind="ExternalInput")
    ident_d = nc.dram_tensor("ident", [128, 128], F32, kind="ExternalInput")
    bo_d = nc.dram_tensor("bo_rep", [NB, D], F32, kind="ExternalInput")
    out_d = nc.dram_tensor("out", [NB, D], F32, kind="ExternalOutput")

    def chunked_src(ap2d, p=128):
        return ap2d.rearrange("(j p) c -> p j c", p=p)

    def dma_eng(alt):
        return nc.scalar if (o["dma_split"] and alt) else nc.sync

    _copy_ctr = [0]

    def ps_copy(dst, src_ap):
        # PSUM->SBUF copies: Pool can't read PSUM on HW. Alternate DVE/Act.
        _copy_ctr[0] += 1
        if o["copy_split"] and _copy_ctr[0] % 2 == 0:
            nc.scalar.activation(dst, src_ap, AF.Copy)
        else:
            nc.vector.tensor_copy(dst, src_ap)

    with tile.TileContext(nc) as tc:
        with (
            tc.tile_pool(name="const", bufs=1) as const,
            tc.tile_pool(name="perb", bufs=1) as perb,
            tc.tile_pool(name="inp", bufs=2 if o["inp2"] else 1) as inp,
            tc.tile_pool(name="inpr", bufs=1) as inpr,
            tc.tile_pool(name="wqqp", bufs=2 if o["wqq2"] else 1) as wqqp,
            tc.tile_pool(name="wbt", bufs=o["wbt_bufs"]) as wbtp,
            tc.tile_pool(name="ptp", bufs=1) as ptp,
            tc.tile_pool(name="affp", bufs=1) as affp,
            tc.tile_pool(name="hch", bufs=o["hch_bufs"]) as hchp,
            tc.tile_pool(name="prod", bufs=o["prod_bufs"]) as prodp,
            tc.tile_pool(name="wot", bufs=o["wot_bufs"]) as wotp,
            tc.tile_pool(name="psA", bufs=2, space="PSUM") as psA,
            tc.tile_pool(name="psB", bufs=2, space="PSUM") as psB,
            tc.tile_pool(name="psC", bufs=o["psC_bufs"], space="PSUM") as psC,
        ):
            # ---- constants (outside repeat loop) ----
            wqt = const.tile([128, ET * K], F32R, tag="wqt")
            wvt = const.tile([128, ET * K], F32R, tag="wvt")
            EH2 = ET // 2
            nc.sync.dma_start(
                wqt[:, 0:EH2 * K].rearrange("p (j c) -> p j c", j=EH2),
                chunked_src(WqT_d[0:EH2 * 128, :]))
            dma_eng(1).dma_start(
                wqt[:, EH2 * K:].rearrange("p (j c) -> p j c", j=EH2),
                chunked_src(WqT_d[EH2 * 128:, :]))
            nc.sync.dma_start(
                wvt[:, 0:EH2 * K].rearrange("p (j c) -> p j c", j=EH2),
                chunked_src(WvT_d[0:EH2 * 128, :]))
            dma_eng(1).dma_start(
                wvt[:, EH2 * K:].rearrange("p (j c) -> p j c", j=EH2),
                chunked_src(WvT_d[EH2 * 128:, :]))
            whvb = const.tile([128, K], F32, tag="whvb")
            whqb = const.tile([128, K], F32, tag="whqb")
            nc.gpsimd.dma_start(whvb[:], WhvB_d[:])
            nc.gpsimd.dma_start(whqb[:], WhqB_d[:])
            ident = const.tile([128, 128], F32, tag="ident")
            nc.gpsimd.dma_start(ident[:], ident_d[:])
            bo_t = const.tile([NB, D], F32, tag="bo")
            nc.gpsimd.dma_start(bo_t[:], bo_d[:])
            # TComb col layout: c*32 + h*4 + b
            tcomb = const.tile([128, ET * H * NB], BF16 if o["wo_bf16"] else F32R, tag="tcomb")

            import contextlib
            loop_cm = tc.For_i(0, repeat, 1) if repeat > 1 else contextlib.nullcontext()
            with loop_cm:
              for b in range(NB):
                  # ---- batched input DMAs ----
                  tet = inp.tile([128, ET * LT], F32R, tag="tet")
                  iet = inp.tile([128, ET * LI], F32R, tag="iet")
                  trows = inpr.tile([128, XT * LA], F32R, tag="trows")
                  irows = inpr.tile([128, YT * LA], F32R, tag="irows")
                  nc.sync.dma_start(
                      tet[:].rearrange("p (j c) -> p j c", j=ET),
                      chunked_src(textT[b]))
                  nc.gpsimd.dma_start(
                      iet[:].rearrange("p (j c) -> p j c", j=ET),
                      chunked_src(imageT[b]))
                  nc.gpsimd.dma_start(
                      trows[:].rearrange("p (j c) -> p j c", j=XT),
                      chunked_src(text_aug[b]))
                  nc.sync.dma_start(
                      irows[:, 0:4 * LA].rearrange("p (j c) -> p j c", j=4),
                      chunked_src(image_aug[b, 0:512, :]))
                  nc.sync.dma_start(
                      irows[0:64, 4 * LA:5 * LA], image_aug[b, 512:576, :])

                  # ---- wq_q [x,k] ----
                  wqq = wqqp.tile([128, XT * K], F32R, tag="wqq")
                  for i in range(XT):
                      ps = psA.tile([128, K], F32, tag="mm1")
                      for j in range(ET):
                          nc.tensor.matmul(
                              ps[:], tet[:, j * LT + i * 128: j * LT + (i + 1) * 128],
                              wqt[:, j * K:(j + 1) * K],
                              start=(j == 0), stop=(j == ET - 1))
                      ps_copy(wqq[:, i * K:(i + 1) * K], ps[:])

                  # ---- wv_v [y,k] ----
                  wvv = perb.tile([128, YT * K], F32R, tag="wvv")
                  for j_y in range(YT):
                      p = ycols(j_y)
                      ps = psA.tile([128, K], F32, tag="mm1")
                      for j in range(ET):
                          nc.tensor.matmul(
                              ps[0:p, :],
                              iet[:, j * LI + j_y * 128: j * LI + j_y * 128 + p],
                              wvt[:, j * K:(j + 1) * K],
                              start=(j == 0), stop=(j == ET - 1))
                      ps_copy(wvv[0:p, j_y * K:(j_y + 1) * K], ps[0:p, :])

                  # ---- G_v [e,k] = image.T @ wv_v ----
                  gv = perb.tile([128, ET * K], F32R, tag="gv")
                  for c in range(ET):
                      ps = psA.tile([128, K], F32, tag="mm1")
                      for j_y in range(YT):
                          p = ycols(j_y)
                          nc.tensor.matmul(
                              ps[:],
                              irows[0:p, j_y * LA + c * 128: j_y * LA + (c + 1) * 128],
                              wvv[0:p, j_y * K:(j_y + 1) * K],
                              start=(j_y == 0), stop=(j_y == YT - 1))
                      ps_copy(gv[:, c * K:(c + 1) * K], ps[:])

                  svq = perb.tile([128, (YT + XT) * H], F32, tag="svq")
                  sv = svq[:, 0:YT * H]
                  sq = svq[:, YT * H:(YT + XT) * H]
                  nc.vector.memset(svq[:], 0.0)

                  # ---- heads ----
                  for h in range(H):
                      if o["wbt_half"]:
                          EH = ET // 2
                          wbts = []
                          for half in range(2):
                              wb = wbtp.tile([128, EH * D], F32R, tag="wbt")
                              nc.sync.dma_start(
                                  wb[:].rearrange("p (j c) -> p j c", j=EH),
                                  chunked_src(WbT_d[h, half * EH * 128:(half + 1) * EH * 128, :]))
                              wbts.append(wb)

                          def wbt_sl(j, c):
                              return wbts[j // EH][:, (j % EH) * D + c * 128: (j % EH) * D + (c + 1) * 128]
                      else:
                          wbt = wbtp.tile([128, ET * D], F32R, tag="wbt")
                          nc.sync.dma_start(
                              wbt[:].rearrange("p (j c) -> p j c", j=ET),
                              chunked_src(WbT_d[h]))

                          def wbt_sl(j, c):
                              return wbt[:, j * D + c * 128: j * D + (c + 1) * 128]

                      # ptT [f,x] = Wb[h] @ text.T
                      ptt = ptp.tile([128, ET * LT], F32R, tag="ptt")
                      for c in range(ET):
                          ps = psA.tile([128, LT], F32, tag="mm1")
                          for j in range(ET):
                              nc.tensor.matmul(
                                  ps[:], wbt_sl(j, c),
                                  tet[:, j * LT:(j + 1) * LT],
                                  start=(j == 0), stop=(j == ET - 1))
                          ps_copy(ptt[:, c * LT:(c + 1) * LT], ps[:])

                      # aff [x,y] = pt @ image.T   (y split 288+288)
                      aft = affp.tile([128, XT * LI], F32R, tag="aff")
                      for i in range(XT):
                          ps = psB.tile([128, 1024], F32, tag="aff")
                          for j in range(ET):
                              lhs = ptt[:, j * LT + i * 128: j * LT + (i + 1) * 128]
                              nc.tensor.matmul(
                                  ps[:, 0:288], lhs,
                                  iet[:, j * LI: j * LI + 288],
                                  start=(j == 0), stop=(j == ET - 1))
                              nc.tensor.matmul(
                                  ps[:, 512:800], lhs,
                                  iet[:, j * LI + 288: j * LI + 576],
                                  start=(j == 0), stop=(j == ET - 1))
                          ps_copy(
                              aft[:, i * LI: i * LI + 288], ps[:, 0:288])
                          ps_copy(
                              aft[:, i * LI + 288: i * LI + 576], ps[:, 512:800])

                      # wqqc [y,k] + wv_v -> tanh -> *Whv -> reduce -> S_v
                      for j_y in range(YT):
                          p = ycols(j_y)
                          ps = psC.tile([128, K], F32, tag="pre")
                          for i in range(XT):
                              nc.tensor.matmul(
                                  ps[0:p, :],
                                  aft[:, i * LI + j_y * 128: i * LI + j_y * 128 + p],
                                  wqq[:, i * K:(i + 1) * K],
                                  start=(i == 0), stop=(i == XT - 1))
                          pre = hchp.tile([128, K], F32, tag="hch")
                          nc.vector.tensor_tensor(
                              out=pre[0:p, :], in0=ps[0:p, :],
                              in1=wvv[0:p, j_y * K:(j_y + 1) * K].bitcast(F32), op=OP.add)
                          hc = prodp.tile([128, K], F32, tag="prod")
                          nc.scalar.activation(hc[0:p, :], pre[0:p, :], AF.Tanh)
                          nc.gpsimd.tensor_tensor(
                              out=pre[0:p, :], in0=hc[0:p, :], in1=whvb[0:p, :], op=OP.mult)
                          nc.vector.tensor_reduce(
                              sv[0:p, j_y * H + h: j_y * H + h + 1], pre[0:p, :],
                              axis=mybir.AxisListType.X, op=OP.add)

                      # wvvc [x,k] + wq_q -> tanh -> *Whq -> reduce -> S_q
                      for i in range(XT):
                          ps = psC.tile([128, K], F32, tag="pre")
                          for c in range(ET):
                              nc.tensor.matmul(
                                  ps[:],
                                  ptt[:, c * LT + i * 128: c * LT + (i + 1) * 128],
                                  gv[:, c * K:(c + 1) * K],
                                  start=(c == 0), stop=(c == ET - 1))
                          pre = hchp.tile([128, K], F32, tag="hch")
                          nc.vector.tensor_tensor(
                              out=pre[:], in0=ps[:],
                              in1=wqq[:, i * K:(i + 1) * K].bitcast(F32), op=OP.add)
                          hc = prodp.tile([128, K], F32, tag="prod")
                          nc.scalar.activation(hc[:], pre[:], AF.Tanh)
                          nc.gpsimd.tensor_tensor(out=pre[:], in0=hc[:], in1=whqb[:], op=OP.mult)
                          nc.vector.tensor_reduce(
                              sq[:, i * H + h: i * H + h + 1], pre[:],
                              axis=mybir.AxisListType.X, op=OP.add)

                  # ---- softmax numerators (no max-sub; logits bounded) ----
                  evqr = perb.tile([128, (YT + XT) * H], F32R, tag="evqr")
                  evr = evqr[:, 0:YT * H]
                  eqr = evqr[:, YT * H:(YT + XT) * H]
                  nc.scalar.activation(evqr[:], svq[:], AF.Exp)

                  # ---- ctx_v = E_v.T @ [image|1] ; ctx_q = E_q.T @ [text|1] ----
                  ho = perb.tile([H, D], F32, tag="ho")
                  tmph = perb.tile([H, D], F32, tag="tmph")
                  for (er, rows, nt, dest) in (
                      (evr, irows, YT, ho),
                      (eqr, trows, XT, tmph),
                  ):
                      c512 = psA.tile([H, 512], F32, tag="mm1")
                      c257 = psA.tile([H, 260], F32, tag="mm1")
                      for j in range(nt):
                          p = ycols(j) if nt == YT else 128
                          lhs = er[0:p, j * H:(j + 1) * H]
                          nc.tensor.matmul(c512[:], lhs, rows[0:p, j * LA: j * LA + 512],
                                           start=(j == 0), stop=(j == nt - 1))
                          nc.tensor.matmul(c257[:], lhs, rows[0:p, j * LA + 512: j * LA + 772],
                                           start=(j == 0), stop=(j == nt - 1))
                      rcp = perb.tile([H, 1], F32, tag="rcp" + ("v" if dest is ho else "q"))
                      nc.vector.reciprocal(rcp[:], c257[:, 256:257])
                      nc.scalar.activation(dest[:, 0:512], c512[:], AF.Copy, scale=rcp[:])
                      nc.scalar.activation(dest[:, 512:768], c257[:, 0:256], AF.Copy, scale=rcp[:])

                  # ---- transpose head_out into TComb (col = c*32 + h*4 + b) ----
                  tc3 = tcomb[:].rearrange("p (c h b) -> p c h b", c=ET, h=H)
                  for c in range(ET):
                      pst = psA.tile([128, H], F32, tag="mm1")
                      nc.tensor.matmul(pst[:], ho[:, c * 128:(c + 1) * 128], ident[0:H, 0:H],
                                       is_transpose=True, start=True, stop=False)
                      nc.tensor.matmul(pst[:], tmph[:, c * 128:(c + 1) * 128], ident[0:H, 0:H],
                                       is_transpose=True, start=False, stop=True)
                      nc.vector.tensor_copy(tc3[:, c, :, b], pst[:])

              # ---- final: out = Wo @ combined + bo ----
              out_t = perb.tile([NB, D], F32, tag="outt")
              f512 = psA.tile([NB, 512], F32, tag="mm1")
              f256 = psA.tile([NB, 256], F32, tag="mm1")
              tc3 = tcomb[:].rearrange("p (c h b) -> p c h b", c=ET, h=H)
              WOB = o["wob"]  # WoT tiles per DMA
              for tb in range(H * ET // WOB):
                  wo = wotp.tile([128, WOB * D], wo_dt, tag="wot")
                  engs = ([nc.sync, nc.sync, nc.sync, nc.scalar, nc.scalar, nc.gpsimd]
                          if o["wo_pool_dma"] else [nc.sync, nc.scalar])
                  engs[tb % len(engs)].dma_start(
                      wo[:].rearrange("p (q c) -> p q c", q=WOB),
                      chunked_src(WoT_d[tb * WOB * 128:(tb + 1) * WOB * 128, :]))
                  for q in range(WOB):
                      t = tb * WOB + q
                      h, c = t // ET, t % ET
                      lhs = tc3[:, c, h, :]
                      nc.tensor.matmul(f512[:], lhs, wo[:, q * D: q * D + 512],
                                       start=(t == 0), stop=(t == H * ET - 1))
                      nc.tensor.matmul(f256[:], lhs, wo[:, q * D + 512:(q + 1) * D],
                                       start=(t == 0), stop=(t == H * ET - 1))
              nc.vector.tensor_tensor(out=out_t[:, 0:512], in0=f512[:], in1=bo_t[:, 0:512], op=OP.add)
              nc.vector.tensor_tensor(out=out_t[:, 512:768], in0=f256[:], in1=bo_t[:, 512:768], op=OP.add)
              nc.sync.dma_start(out_d[:], out_t[:])

    nc.compile()
    return nc


_nc_cache = None


def _get_nc():
    global _nc_cache
    if _nc_cache is None:
        _nc_cache = build_nc()
    return _nc_cache


WO_BF16 = True


def make_in_maps(inputs):
    return _make_in_maps(**inputs)


def _make_in_maps(text_hidden_states, image_hidden_states, text_mask, Wb, Wv,
                  Wq, Whv, Whq, Wo, bo, **_unused):
    text = np.ascontiguousarray(np.asarray(text_hidden_states, np.float32))
    image = np.ascontiguousarray(np.asarray(image_hidden_states, np.float32))
    Wb = np.asarray(Wb, np.float32)
    Wv = np.asarray(Wv, np.float32)
    Wq = np.asarray(Wq, np.float32)
    Whv = np.asarray(Whv, np.float32)
    Whq = np.asarray(Whq, np.float32)
    Wo = np.asarray(Wo, np.float32)
    bo = np.asarray(bo, np.float32)

    WqT = np.ascontiguousarray(Wq.T)
    WvT = np.ascontiguousarray(Wv.T)
    WbT = np.ascontiguousarray(np.transpose(Wb, (0, 2, 1)))
    WoT = np.ascontiguousarray(Wo.T)
    if WO_BF16:
        import ml_dtypes
        WoT = WoT.astype(ml_dtypes.bfloat16)
    WhvB = np.ascontiguousarray(np.broadcast_to(Whv[None, :], (128, K)))
    WhqB = np.ascontiguousarray(np.broadcast_to(Whq[None, :], (128, K)))
    ident = np.eye(128, dtype=np.float32)

    textT = np.ascontiguousarray(np.transpose(text, (0, 2, 1)))
    imageT = np.ascontiguousarray(np.transpose(image, (0, 2, 1)))
    pad_t = np.zeros((B, LT, 4), np.float32); pad_t[:, :, 0] = 1.0
    pad_i = np.zeros((B, LI, 4), np.float32); pad_i[:, :, 0] = 1.0
    text_aug = np.concatenate([text, pad_t], axis=2)
    image_aug = np.concatenate([image, pad_i], axis=2)

    in_maps = []
    for c in range(N_CORES):
        sl = slice(c * NB, (c + 1) * NB)
        in_maps.append({
            "textT": textT[sl], "text_aug": text_aug[sl],
            "imageT": imageT[sl], "image_aug": image_aug[sl],
            "WqT": WqT, "WvT": WvT, "WbT": WbT,
            "WhvB": WhvB, "WhqB": WhqB, "WoT": WoT,
            "ident": ident,
            "bo_rep": np.ascontiguousarray(np.broadcast_to(bo[None, :], (NB, D))),
        })
    return in_maps


def kernel(**inputs):
    nc = _get_nc()
    in_maps = make_in_maps(inputs)
    r = run_bass_kernel_spmd(nc, in_maps, list(range(N_CORES)))
    return np.concatenate([r.results[c]["out"] for c in range(N_CORES)], axis=0)
